# revision 1
# baseline (speedup 1.0000x reference)
"""GNN linear-attention kernel for Trainium2 (8 NeuronCores).

Sharding: data-parallel over batch B=8 -- one graph (N=2048 nodes) per
NeuronCore; parameters replicated. Inputs are full (unsharded) numpy
arrays; output is the full (B, N, O) float32 array.
"""

import numpy as np

B, N, D, O = 8, 2048, 128, 128

_compiled = {}


def _get_fn():
    import jax
    import jax.numpy as jnp

    if "fn" in _compiled:
        return _compiled["fn"]

    def f(x_b, A_u8, W_qk, b_qk, W_l, b_l, W_r, W_d, b_d):
        d = x_b.shape[-1]
        A_b = A_u8.astype(jnp.float32)
        deg = jnp.sum(A_b, axis=-1, keepdims=True)
        gate = jax.nn.sigmoid(deg @ W_d + b_d)
        xg = x_b * gate
        QK = jax.nn.sigmoid(xg @ W_qk + b_qk)
        scores = (QK @ QK.T) / jnp.sqrt(jnp.float32(d))
        scores = scores * A_b
        attn = scores / (jnp.sum(scores, axis=-1, keepdims=True) + 1e-6)
        agg = attn @ xg
        out = agg @ W_l + b_l + xg @ W_r
        nrm = jnp.linalg.norm(out, axis=-1, keepdims=True)
        return out / jnp.maximum(nrm, 1e-12)

    _compiled["fn"] = jax.jit(f)
    _compiled["pfn"] = jax.pmap(
        f, in_axes=(0, 0, None, None, None, None, None, None, None)
    )
    return _compiled["fn"]


def _run_loop(x, A, weights):
    import jax

    fn = _get_fn()
    devs = jax.devices()
    ndev = min(len(devs), x.shape[0])
    futs = []
    for b in range(x.shape[0]):
        dev = devs[b % ndev]
        args = [jax.device_put(np.asarray(t), dev) for t in (x[b], A[b]) + weights]
        futs.append(fn(*args))
    return np.stack([np.asarray(f_) for f_ in futs], axis=0)


def kernel(x, A, W_qk, b_qk, W_l, b_l, W_r, W_d, b_d):
    weights = (W_qk, b_qk, W_l, b_l, W_r, W_d, b_d)
    _get_fn()
    # A is a 0/1 adjacency matrix: ship it as uint8 (lossless, 4x less
    # transfer) and cast back to f32 on-device.
    A_u8 = A.astype(np.uint8)
    try:
        # One parallel dispatch: one graph per NeuronCore.
        out = np.asarray(_compiled["pfn"](x, A_u8, *weights))
    except Exception:
        out = _run_loop(x, A_u8, weights)
    return out.astype(np.float32)



# revision 2
# speedup vs baseline: 2.0492x; 2.0492x over previous
"""GNN linear-attention kernel for Trainium2 over an axon-tunneled PJRT client.

The device compute for this problem (~2 GFLOP/graph) is trivial next to the
cost of moving data through the tunnel (~30-60 MB/s, ~90 ms/RPC), so the
kernel is organized entirely around the data path:

  - A is a 0/1 adjacency matrix: pack to 1 bit/element on the host
    (134 MB f32 -> 4.2 MB) and unpack on-device.
  - x ships as f16 (4.2 MB), the output returns as f16 and is upcast on the
    host (rel-err ~2e-3, well under the 2e-2 gate).
  - The batch is split into chunks; uploads, device execution, and downloads
    of different chunks overlap via threads (the tunnel is full-duplex).
  - Everything runs on one NeuronCore: per-op RPC latency dominates any
    benefit of spreading trivial compute across 8 cores.
"""

import math
import numpy as np
from concurrent.futures import ThreadPoolExecutor

B, N, D, O = 8, 2048, 128, 128
NCHUNKS = 2
CHUNK = B // NCHUNKS

_state = {}


def _get_state():
    if _state:
        return _state
    import jax
    import jax.numpy as jnp

    dev = jax.devices()[0]

    def parse_weights(wbuf):
        i = 0

        def take(n, shape):
            nonlocal i
            t = wbuf[i : i + n].reshape(shape)
            i += n
            return t

        W_qk = take(D * D, (D, D))
        b_qk = take(D, (D,))
        W_l = take(D * O, (D, O))
        b_l = take(O, (O,))
        W_r = take(D * O, (D, O))
        W_d = take(D, (1, D))
        b_d = take(D, (D,))
        return W_qk, b_qk, W_l, b_l, W_r, W_d, b_d

    def graph_compute(bits, x16, wbuf):
        # bits: (g, N, N//8) u8, x16: (g, N, D) f16, wbuf: flat f32
        W_qk, b_qk, W_l, b_l, W_r, W_d, b_d = parse_weights(wbuf)
        shifts = jnp.arange(7, -1, -1, dtype=jnp.uint8)
        A = ((bits[..., None] >> shifts) & jnp.uint8(1))
        A = A.reshape(bits.shape[0], N, N).astype(jnp.float32)
        x = x16.astype(jnp.float32)
        deg = jnp.sum(A, axis=-1, keepdims=True)
        gate = jax.nn.sigmoid(deg @ W_d + b_d)
        xg = x * gate
        QK = jax.nn.sigmoid(xg @ W_qk + b_qk)
        scores = jnp.einsum("bnd,bmd->bnm", QK, QK) / jnp.sqrt(jnp.float32(D))
        scores = scores * A
        attn = scores / (jnp.sum(scores, axis=-1, keepdims=True) + 1e-6)
        agg = jnp.einsum("bnm,bmd->bnd", attn, xg)
        out = agg @ W_l + b_l + xg @ W_r
        nrm = jnp.linalg.norm(out, axis=-1, keepdims=True)
        return (out / jnp.maximum(nrm, 1e-12)).astype(jnp.float16)

    _state["jax"] = jax
    _state["dev"] = dev
    _state["fn"] = jax.jit(graph_compute)
    _state["pool"] = ThreadPoolExecutor(max_workers=8)
    return _state


def _pack_bits(A_chunk):
    # (g, N, N) f32 of 0/1 -> (g, N, N//8) u8, MSB-first like np.packbits
    nz = A_chunk.view(np.uint32) != 0
    return np.packbits(nz, axis=-1)


def _fast_path(x, A, W_qk, b_qk, W_l, b_l, W_r, W_d, b_d):
    st = _get_state()
    jax, dev, fn, pool = st["jax"], st["dev"], st["fn"], st["pool"]

    wbuf = np.concatenate(
        [
            np.ascontiguousarray(W_qk, np.float32).reshape(-1),
            np.ascontiguousarray(b_qk, np.float32).reshape(-1),
            np.ascontiguousarray(W_l, np.float32).reshape(-1),
            np.ascontiguousarray(b_l, np.float32).reshape(-1),
            np.ascontiguousarray(W_r, np.float32).reshape(-1),
            np.ascontiguousarray(W_d, np.float32).reshape(-1),
            np.ascontiguousarray(b_d, np.float32).reshape(-1),
        ]
    )

    def put(arr):
        return jax.device_put(arr, dev)

    # Issue the weight + x uploads immediately; pack A on the main thread
    # (numpy releases the GIL, so packing overlaps the in-flight puts).
    w_fut = pool.submit(put, wbuf)
    x_futs = [
        pool.submit(put, x[c * CHUNK : (c + 1) * CHUNK].astype(np.float16))
        for c in range(NCHUNKS)
    ]
    bits_futs = []
    for c in range(NCHUNKS):
        bits = _pack_bits(A[c * CHUNK : (c + 1) * CHUNK])
        bits_futs.append(pool.submit(put, bits))

    w_dev = w_fut.result()
    outs = []
    for c in range(NCHUNKS):
        y = fn(bits_futs[c].result(), x_futs[c].result(), w_dev)
        try:
            y.copy_to_host_async()
        except Exception:
            pass
        outs.append(y)

    result = np.empty((B, N, O), np.float32)
    for c in range(NCHUNKS):
        result[c * CHUNK : (c + 1) * CHUNK] = np.asarray(outs[c]).astype(np.float32)
    return result


def _fallback(x, A, W_qk, b_qk, W_l, b_l, W_r, W_d, b_d):
    import jax
    import jax.numpy as jnp

    if "fb" not in _state:

        def f(x_b, A_b, W_qk, b_qk, W_l, b_l, W_r, W_d, b_d):
            deg = jnp.sum(A_b, axis=-1, keepdims=True)
            gate = jax.nn.sigmoid(deg @ W_d + b_d)
            xg = x_b * gate
            QK = jax.nn.sigmoid(xg @ W_qk + b_qk)
            scores = (QK @ QK.T) / jnp.sqrt(jnp.float32(D))
            scores = scores * A_b
            attn = scores / (jnp.sum(scores, axis=-1, keepdims=True) + 1e-6)
            agg = attn @ xg
            out = agg @ W_l + b_l + xg @ W_r
            nrm = jnp.linalg.norm(out, axis=-1, keepdims=True)
            return out / jnp.maximum(nrm, 1e-12)

        _state["fb"] = jax.jit(f)
    fn = _state["fb"]
    dev = jax.devices()[0]
    ws = [jax.device_put(np.asarray(t), dev) for t in (W_qk, b_qk, W_l, b_l, W_r, W_d, b_d)]
    out = np.stack(
        [np.asarray(fn(jax.device_put(x[b], dev), jax.device_put(A[b], dev), *ws)) for b in range(B)]
    )
    return out.astype(np.float32)


def kernel(x, A, W_qk, b_qk, W_l, b_l, W_r, W_d, b_d):
    x = np.ascontiguousarray(x, np.float32)
    A = np.ascontiguousarray(A, np.float32)
    try:
        return _fast_path(x, A, W_qk, b_qk, W_l, b_l, W_r, W_d, b_d)
    except Exception:
        return _fallback(x, A, W_qk, b_qk, W_l, b_l, W_r, W_d, b_d)


# revision 3
# speedup vs baseline: 2.8920x; 1.4113x over previous
"""GNN linear-attention kernel for Trainium2 over an axon-tunneled PJRT client.

The device compute for this problem (~2 GFLOP/graph) is trivial next to the
cost of moving data through the tunnel (~30-60 MB/s, ~90 ms/RPC), so the
kernel is organized entirely around the data path:

  - A is a 0/1 adjacency matrix: pack to 1 bit/element on the host
    (134 MB f32 -> 4.2 MB) and unpack on-device.
  - x and the weights ship as one f16 payload per chunk; the output returns
    as f16 and is upcast on the host (rel-err ~3e-4, gate is 2e-2).
  - The batch is split into chunks; uploads, device execution, and downloads
    of different chunks overlap via threads (the tunnel is full-duplex).
  - Everything runs on one NeuronCore: per-op RPC latency dominates any
    benefit of spreading trivial compute across 8 cores.
"""

import numpy as np
from concurrent.futures import ThreadPoolExecutor

B, N, D, O = 8, 2048, 128, 128
NCHUNKS = 4
CHUNK = B // NCHUNKS
NW = 3 * D * D + 4 * D  # f16 elements of packed weights per chunk payload

_state = {}


def _get_state():
    if _state:
        return _state
    import jax
    import jax.numpy as jnp

    dev = jax.devices()[0]

    def chunk_compute(bits, xw):
        # bits: (g, N, N//8) u8;  xw: flat f16 = [weights | x chunk]
        i = 0

        def take(n, shape):
            nonlocal i
            t = xw[i : i + n].astype(jnp.float32).reshape(shape)
            i += n
            return t

        W_qk = take(D * D, (D, D))
        W_l = take(D * O, (D, O))
        W_r = take(D * O, (D, O))
        b_qk = take(D, (D,))
        b_l = take(O, (O,))
        W_d = take(D, (1, D))
        b_d = take(D, (D,))
        x = xw[i:].astype(jnp.float32).reshape(CHUNK, N, D)

        shifts = jnp.arange(7, -1, -1, dtype=jnp.uint8)
        A = (bits[..., None] >> shifts) & jnp.uint8(1)
        A = A.reshape(CHUNK, N, N).astype(jnp.float32)
        deg = jnp.sum(A, axis=-1, keepdims=True)
        gate = jax.nn.sigmoid(deg @ W_d + b_d)
        xg = x * gate
        QK = jax.nn.sigmoid(xg @ W_qk + b_qk)
        scores = jnp.einsum("bnd,bmd->bnm", QK, QK) / jnp.sqrt(jnp.float32(D))
        scores = scores * A
        attn = scores / (jnp.sum(scores, axis=-1, keepdims=True) + 1e-6)
        agg = jnp.einsum("bnm,bmd->bnd", attn, xg)
        out = agg @ W_l + b_l + xg @ W_r
        nrm = jnp.linalg.norm(out, axis=-1, keepdims=True)
        return (out / jnp.maximum(nrm, 1e-12)).astype(jnp.float16)

    _state["jax"] = jax
    _state["dev"] = dev
    _state["fn"] = jax.jit(chunk_compute)
    _state["pool"] = ThreadPoolExecutor(max_workers=8)
    return _state


def _fast_path(x, A, W_qk, b_qk, W_l, b_l, W_r, W_d, b_d):
    st = _get_state()
    jax, dev, fn, pool = st["jax"], st["dev"], st["fn"], st["pool"]

    w16 = np.concatenate(
        [
            np.ascontiguousarray(W_qk, np.float32).reshape(-1),
            np.ascontiguousarray(W_l, np.float32).reshape(-1),
            np.ascontiguousarray(W_r, np.float32).reshape(-1),
            np.ascontiguousarray(b_qk, np.float32).reshape(-1),
            np.ascontiguousarray(b_l, np.float32).reshape(-1),
            np.ascontiguousarray(W_d, np.float32).reshape(-1),
            np.ascontiguousarray(b_d, np.float32).reshape(-1),
        ]
    ).astype(np.float16)
    assert w16.size == NW

    def put(arr):
        return jax.device_put(arr, dev)

    # A viewed as bytes: a 0/1 f32 element is nonzero exactly in its top byte,
    # and np.packbits packs any-nonzero as 1, so pack the strided byte view
    # directly (no bool temp).
    Ab = A.view(np.uint8).reshape(B, N, N, 4)

    ys = []
    fetches = []

    def fetch(y):
        return np.asarray(y).astype(np.float32)

    for c in range(B // CHUNK):
        sl = slice(c * CHUNK, (c + 1) * CHUNK)
        xw = np.concatenate([w16, x[sl].astype(np.float16).reshape(-1)])
        xw_fut = pool.submit(put, xw)
        bits = np.packbits(Ab[sl, :, :, 3], axis=-1)
        bits_fut = pool.submit(put, bits)
        y = fn(bits_fut.result(), xw_fut.result())
        try:
            y.copy_to_host_async()
        except Exception:
            pass
        ys.append(y)
        fetches.append(pool.submit(fetch, y))

    result = np.empty((B, N, O), np.float32)
    for c, f in enumerate(fetches):
        result[c * CHUNK : (c + 1) * CHUNK] = f.result()
    return result


def _fallback(x, A, W_qk, b_qk, W_l, b_l, W_r, W_d, b_d):
    import jax
    import jax.numpy as jnp

    if "fb" not in _state:

        def f(x_b, A_b, W_qk, b_qk, W_l, b_l, W_r, W_d, b_d):
            deg = jnp.sum(A_b, axis=-1, keepdims=True)
            gate = jax.nn.sigmoid(deg @ W_d + b_d)
            xg = x_b * gate
            QK = jax.nn.sigmoid(xg @ W_qk + b_qk)
            scores = (QK @ QK.T) / jnp.sqrt(jnp.float32(D))
            scores = scores * A_b
            attn = scores / (jnp.sum(scores, axis=-1, keepdims=True) + 1e-6)
            agg = attn @ xg
            out = agg @ W_l + b_l + xg @ W_r
            nrm = jnp.linalg.norm(out, axis=-1, keepdims=True)
            return out / jnp.maximum(nrm, 1e-12)

        _state["fb"] = jax.jit(f)
    fn = _state["fb"]
    dev = jax.devices()[0]
    ws = [jax.device_put(np.asarray(t), dev) for t in (W_qk, b_qk, W_l, b_l, W_r, W_d, b_d)]
    out = np.stack(
        [np.asarray(fn(jax.device_put(x[b], dev), jax.device_put(A[b], dev), *ws)) for b in range(B)]
    )
    return out.astype(np.float32)


def kernel(x, A, W_qk, b_qk, W_l, b_l, W_r, W_d, b_d):
    x = np.ascontiguousarray(x, np.float32)
    A = np.ascontiguousarray(A, np.float32)
    try:
        return _fast_path(x, A, W_qk, b_qk, W_l, b_l, W_r, W_d, b_d)
    except Exception:
        return _fallback(x, A, W_qk, b_qk, W_l, b_l, W_r, W_d, b_d)
